# revision 27
# baseline (speedup 1.0000x reference)
"""Trainium2 Bass kernel for nn_ProbAttention (sparse attention / Informer ProbSparse).

Strategy (8 NeuronCores, no collectives):
  core c -> (batch b = c//2, half h = c%2).
  Both cores of a pair compute QK / M for their batch (a pair AllGather
  measured ~35us -- slower than the duplicated compute); the attention
  update and the big Wfin product are column-split: each core only attends
  the selected queries that land in its 512-column shard.

Device pipeline per core (one batch, bf16 PE path; max 2 top-140 selection
swaps vs the fp32 reference on this dataset, rel err ~3e-3 << 2e-2):
  B. K^T, Q^T (bf16) from X^T/W bf16; V(+ones col), vmean, Wadd residual.
  C. QK into PSUM (bf16 matmuls) + additive -30000 sample mask accumulated
     on the PE (ident @ am); DVE reduce-max -> maxacc and fused
     scalar_tensor_tensor (qk/N * cnt, sum) -> sumacc. M = max - sum.
  D. No index compaction at all: M row broadcast via PE (transpose +
     ones-row matmuls), rank[q] = #{j: M[j] > M[q]} for own-half queries
     (4 DVE ops), selm = rank < 140, and the scatter one-hots
     D[q, col] = (col == q) * selm[q] built by one fused tensor_scalar per
     128-query chunk. No DRAM roundtrips, no gpsimd.
  E. scores^T = K^T-slices @ Q^T(own half) for ALL 512 own queries; exp on
     ACT; attn@V with a ones-column in V giving denominators for free.
  F. Scatter aug rows + vmean fill into PSUM via D, add precomputed
     residual (+badd), 10 fused multiply-reduce dots against the
     prefetched Wfin shard, partition-reduce by ones-matmul.

kernel(**inputs) is self-contained: host does layout prep only (permutation,
transposes, count masks from index_sample, Wfin reshape, bf16 casts).
"""

import math
import sys

import numpy as np

sys.path.insert(0, "/opt/trn_rl_repo")

import concourse.bass as bass  # noqa: E402
import concourse.bacc as bacc  # noqa: E402
import concourse.tile as tile  # noqa: E402
from concourse import mybir  # noqa: E402
from concourse.bass_utils import run_bass_kernel_spmd  # noqa: E402

import ml_dtypes  # noqa: E402

B, N, D, NCLS, U = 4, 1024, 256, 10, 140
F32 = mybir.dt.float32
BF16 = mybir.dt.bfloat16
F8E5 = mybir.dt.float8e5
F8E4 = mybir.dt.float8e4
ALU = mybir.AluOpType
ACTF = mybir.ActivationFunctionType
NEG = -28672.0  # exact in fp8e5m2


def build_nc(stage=9):
    nc = bacc.Bacc("TRN2", target_bir_lowering=False, debug=False, num_devices=8)

    xt_d = nc.declare_dram_parameter("xt_h", [D, N], BF16, isOutput=False)
    w_d = nc.declare_dram_parameter("w_all_h", [8, 128, D], BF16, isOutput=False)
    am_d = nc.declare_dram_parameter("am_b", [N, N], F8E5, isOutput=False)
    cnt_d = nc.declare_dram_parameter("cnt_b", [N, N], F8E4, isOutput=False)
    wf_d = nc.declare_dram_parameter("wfin_h", [NCLS, 128, N], BF16, isOutput=False)
    misc_d = nc.declare_dram_parameter("misc", [128, 518], F32, isOutput=False)
    id32_d = nc.declare_dram_parameter("ident32", [128, 128], F32, isOutput=False)
    idb_d = nc.declare_dram_parameter("identb", [128, 128], F8E5, isOutput=False)
    sel8_d = nc.declare_dram_parameter("sel8", [8, 1024], F32, isOutput=False)
    out_d = nc.declare_dram_parameter("out10", [1, 16], F32, isOutput=True)

    def emit(tc):
        with (
            tc.tile_pool(name="const", bufs=1) as cpool,
            tc.tile_pool(name="wstream", bufs=1) as wpool,
            tc.tile_pool(name="big", bufs=1) as bpool,
            tc.tile_pool(name="maskA", bufs=3) as mpoolA,
            tc.tile_pool(name="maskB", bufs=3) as mpoolB,
            tc.tile_pool(name="scrA", bufs=2) as spoolA,
            tc.tile_pool(name="scrB", bufs=2) as spoolB,
            tc.tile_pool(name="small", bufs=1) as smpool,
        ):
            # ---- constant loads + memset consts ----
            xt = [cpool.tile([128, N], BF16, name=f"xt{i}", tag=f"xt{i}") for i in range(2)]
            for ft in range(2):
                nc.sync.dma_start(xt[ft][:], xt_d[ft * 128:(ft + 1) * 128, :])
            wall = cpool.tile([128, 8 * D], BF16, name="wall", tag="wall")
            nc.sync.dma_start(
                wall[:], bass.AP(w_d, 0, [[D, 128], [128 * D, 8], [1, D]])
            )
            wrb = {nm: [wall[:, (2 * i + ft) * D:(2 * i + ft + 1) * D] for ft in range(2)]
                   for i, nm in enumerate(("q", "k", "v", "a"))}
            misc = cpool.tile([128, 518], F32, name="misc", tag="misc")
            nc.sync.dma_start(misc[:], misc_d[:, :])
            nrow = misc[:, 0:512]
            qiota = misc[:, 512:516]
            badd = [misc[:, 516 + i:517 + i] for i in range(2)]
            ident32 = cpool.tile([128, 128], F32, name="ident32", tag="ident32")
            nc.sync.dma_start(ident32[:], id32_d[:, :])
            identb = cpool.tile([128, 128], F8E5, name="identb", tag="identb")
            nc.sync.dma_start(identb[:], idb_d[:, :])
            sel8 = cpool.tile([8, 1024], F32, name="sel8", tag="sel8")
            nc.sync.dma_start(sel8[:], sel8_d[:, :])
            onesrow32 = cpool.tile([1, 128], F32, name="onesrow32", tag="onesrow32")
            nc.gpsimd.memset(onesrow32[:], 1.0)
            ones16 = cpool.tile([128, 1], BF16, name="ones16", tag="ones16")
            nc.gpsimd.memset(ones16[:], 1.0)
            onesr32 = cpool.tile([128, 1], F32, name="onesr32", tag="onesr32")
            nc.gpsimd.memset(onesr32[:], 1.0)
            # wf tiles allocated here; their DMAs are issued after the mask
            # loads (emission order = sync-queue order) so the critical
            # phase-C streams aren't starved by the prefetch.
            wf = [wpool.tile([128, N], BF16, name=f"wf{c}", tag=f"wf{c}")
                  for c in range(NCLS)]

            # ---- phase B1: K^T / Q^T in bf16 ----
            ktT = [bpool.tile([128, N], BF16, name=f"ktT{i}", tag=f"ktT{i}") for i in range(2)]
            qtT = [bpool.tile([128, N], BF16, name=f"qtT{i}", tag=f"qtT{i}") for i in range(2)]
            vnp = [bpool.tile([128, D + 1], BF16, name=f"vnp{i}", tag=f"vnp{i}") for i in range(8)]
            for kt in range(8):
                nc.gpsimd.memset(vnp[kt][:, D:D + 1], 1.0)
            vmean_row = smpool.tile([1, D], BF16, tag="vmean_row")
            resid16 = bpool.tile([128, 2 * 512], BF16, name="resid16", tag="resid16")
            maxacc = smpool.tile([128, 8], F32, tag="maxacc")
            sumacc = smpool.tile([128, 8], F32, tag="sumacc")

            with tc.tile_pool(name="psA", bufs=2, space="PSUM") as psA:
                for wt, dst in ((wrb["k"], ktT), (wrb["q"], qtT)):
                    for et in range(2):
                        for nck in range(2):
                            ps = psA.tile([128, 512], F32, tag="psA")
                            for ft in range(2):
                                nc.tensor.matmul(
                                    ps[:],
                                    wt[ft][:, et * 128:(et + 1) * 128],
                                    xt[ft][:, nck * 512:(nck + 1) * 512],
                                    start=(ft == 0), stop=(ft == 1),
                                )
                            nc.scalar.copy(dst[et][:, nck * 512:(nck + 1) * 512], ps[:])

            # ---- phase C: QK + PE mask-add + fused max / sampled sum ----
            with tc.tile_pool(name="psQK", bufs=2, space="PSUM") as psQK:
                for qt in range(8):
                    am = mpoolA.tile([128, N], F8E5, tag="am")
                    nc.sync.dma_start(am[:], am_d[qt * 128:(qt + 1) * 128, :])
                    ct2 = mpoolB.tile([128, N], F8E4, tag="ct2")
                    nc.sync.dma_start(ct2[:], cnt_d[qt * 128:(qt + 1) * 128, :])
                    qk = psQK.tile([128, N], F32, tag="qk")
                    for kc in range(2):
                        for et in range(2):
                            nc.tensor.matmul(
                                qk[:, kc * 512:(kc + 1) * 512],
                                qtT[et][:, qt * 128:(qt + 1) * 128],
                                ktT[et][:, kc * 512:(kc + 1) * 512],
                                start=(et == 0), stop=False,
                            )
                        nc.tensor.matmul(
                            qk[:, kc * 512:(kc + 1) * 512], identb[:],
                            am[:, kc * 512:(kc + 1) * 512],
                            start=False, stop=True,
                        )
                    nc.vector.tensor_reduce(
                        maxacc[:, qt:qt + 1], qk[:], mybir.AxisListType.X, ALU.max
                    )
                    # (qk - 30000)*cnt == qk*cnt at sampled entries (cnt=0 off)
                    scrB = spoolB.tile([128, N], BF16, tag="scrB")
                    nc.vector.scalar_tensor_tensor(
                        scrB[:], qk[:], 1.0 / N, ct2[:], ALU.mult, ALU.mult,
                        accum_out=sumacc[:, qt:qt + 1],
                    )

            for c in range(NCLS):
                nc.sync.dma_start(wf[c][:], wf_d[c, :, :])
            m_sb = smpool.tile([128, 8], F32, tag="m_sb")
            nc.vector.tensor_sub(m_sb[:], maxacc[:], sumacc[:])
            if stage == 1:
                nc.sync.dma_start(out_d[:, 0:8], m_sb[0:1, :])
                return

            # ---- phase D: PE-broadcast M, rank own half, selm, one-hots ----
            rank = smpool.tile([128, 4], F32, tag="rank")
            selm = smpool.tile([128, 4], F32, tag="selm")
            dsel = [smpool.tile([128, 512], BF16, name=f"dsel{i}", tag=f"dsel{i}")
                    for i in range(4)]
            with tc.tile_pool(name="psM", bufs=1, space="PSUM") as psM:
                psT = psM.tile([8, 128], F32, tag="psT")
                nc.tensor.transpose(psT[:], m_sb[:], ident32[:])
                m8 = smpool.tile([8, 128], F32, tag="m8")
                nc.scalar.copy(m8[:], psT[:])
                psm = psM.tile([128, N], F32, tag="psm")
                for r in range(8):
                    nc.tensor.matmul(
                        psm[:, r * 128:(r + 1) * 128],
                        sel8[:, r * 128:(r + 1) * 128], m8[:],
                        start=True, stop=True,
                    )
                # rank split across DVE (is_gt count) and ACT (Sign-sum:
                # Sign(0)=0 and no duplicate M values, so
                # #gt = (sum_j Sign(M[j]-M[q]) + 1023) / 2).
                negm = smpool.tile([128, 2], F32, tag="negm")
                nc.scalar.mul(negm[:], m_sb[:, 2:4], -1.0)
                sgacc = smpool.tile([128, 2], F32, tag="sgacc")
                for qt in range(2):
                    scr = (spoolA if qt < 1 else spoolB).tile([128, N], BF16, tag="scrR")
                    nc.vector.tensor_scalar(
                        scr[:], psm[:], m_sb[:, qt:qt + 1], None, ALU.is_gt,
                        ALU.add, accum_out=rank[:, qt:qt + 1],
                    )
                for qt in range(2, 4):
                    sg = (spoolA if qt < 3 else spoolB).tile([128, N], F32, tag="scrS")
                    nc.scalar.activation(
                        sg[:], psm[:], ACTF.Sign, bias=negm[:, qt - 2:qt - 1],
                        scale=1.0, accum_out=sgacc[:, qt - 2:qt - 1],
                    )
                nc.vector.tensor_scalar(
                    rank[:, 2:4], sgacc[:], 0.5, 511.5, ALU.mult, ALU.add
                )
            nc.vector.tensor_scalar(selm[:], rank[:], 139.5, None, ALU.is_le)
            for qt in range(4):
                nc.vector.tensor_scalar(
                    dsel[qt][:], nrow[:], qiota[:, qt:qt + 1], selm[:, qt:qt + 1],
                    ALU.is_equal, ALU.mult,
                )
            if stage == 2:
                nc.sync.dma_start(out_d[:, 0:4], rank[0:1, :])
                nc.sync.dma_start(out_d[:, 4:8], selm[0:1, :])
                return

            # ---- phase B2 (PE fills rank latency): V, vmean, residual ----
            with tc.tile_pool(name="psB", bufs=2, space="PSUM") as psB:
                for kt in range(8):
                    ps = psB.tile([128, D], F32, tag="psb2")
                    for ft in range(2):
                        nc.tensor.matmul(
                            ps[:], xt[ft][:, kt * 128:(kt + 1) * 128],
                            wrb["v"][ft][:],
                            start=(ft == 0), stop=(ft == 1),
                        )
                    nc.scalar.copy(vnp[kt][:, 0:D], ps[:])
                psvm = psB.tile([1, D], F32, tag="psvm", bufs=1)
                for kt in range(8):
                    nc.tensor.matmul(
                        psvm[:], ones16[:], vnp[kt][:, 0:D],
                        start=(kt == 0), stop=(kt == 7),
                    )
                nc.scalar.mul(vmean_row[:], psvm[:], 1.0 / N)
                for dtl in range(2):
                    ps = psB.tile([128, 512], F32, tag="psrd", bufs=2)
                    for ft in range(2):
                        nc.tensor.matmul(
                            ps[:], wrb["a"][ft][:, dtl * 128:(dtl + 1) * 128],
                            xt[ft][:, 0:512],
                            start=(ft == 0), stop=(ft == 1),
                        )
                    nc.scalar.activation(
                        resid16[:, dtl * 512:(dtl + 1) * 512], ps[:],
                        ACTF.Identity, bias=badd[dtl][:], scale=1.0,
                    )

            # ---- phase E: scores^T for all own-half queries, exp, attn@V ----
            expdT = [bpool.tile([128, 512], BF16, name=f"expdT{i}", tag=f"expdT{i}")
                     for i in range(8)]
            aug = [smpool.tile([128, D], BF16, name=f"aug{i}", tag=f"aug{i}")
                   for i in range(4)]
            ctxh = bpool.tile([128, 2 * 512], BF16, name="ctxh", tag="ctxh")
            facc = smpool.tile([128, 16], F32, tag="facc")
            nc.gpsimd.memset(facc[:, NCLS:16], 0.0)
            with tc.tile_pool(name="psC", bufs=2, space="PSUM") as psC, \
                 tc.tile_pool(name="psE", bufs=2, space="PSUM") as psE:
                for kt in range(8):
                    ps = psC.tile([128, 512], F32, tag="psC")
                    for et in range(2):
                        nc.tensor.matmul(
                            ps[:], ktT[et][:, kt * 128:(kt + 1) * 128],
                            qtT[et][:, 0:512],
                            start=(et == 0), stop=(et == 1),
                        )
                    nc.scalar.activation(
                        expdT[kt][:], ps[:], ACTF.Exp, scale=1.0 / math.sqrt(D)
                    )
                for qc in range(4):
                    pse = psE.tile([128, D + 1], F32, tag="pse")
                    for kt in range(8):
                        nc.tensor.matmul(
                            pse[:], expdT[kt][:, qc * 128:(qc + 1) * 128], vnp[kt][:],
                            start=(kt == 0), stop=(kt == 7),
                        )
                    rc = smpool.tile([128, 1], F32, tag=f"rc{qc}")
                    nc.vector.reciprocal(rc[:], pse[:, D:D + 1])
                    nc.vector.tensor_scalar(
                        aug[qc][:], pse[:, 0:D], rc[:], None, ALU.mult
                    )
            if stage == 4:
                nc.sync.dma_start(out_d[:, :], aug[0][0:1, 0:16].bitcast(BF16))
                return

            # ---- phase F: scatter + fill + residual add + fused dots ----
            with tc.tile_pool(name="psF", bufs=2, space="PSUM") as psF, \
                 tc.tile_pool(name="psCt", bufs=1, space="PSUM") as psCt:
                cnt_ps = psCt.tile([1, 512], F32, tag="cnt_ps")
                for qc in range(4):
                    nc.tensor.matmul(cnt_ps[:], ones16[:], dsel[qc][:],
                                     start=(qc == 0), stop=(qc == 3))
                fill16 = smpool.tile([1, 512], BF16, tag="fill16")
                nc.scalar.activation(fill16[:], cnt_ps[:], ACTF.Copy,
                                     bias=1.0, scale=-1.0)
                for dtl in range(2):
                    ps = psF.tile([128, 512], F32, tag="psF")
                    for qc in range(4):
                        nc.tensor.matmul(
                            ps[:], aug[qc][:, dtl * 128:(dtl + 1) * 128], dsel[qc][:],
                            start=(qc == 0), stop=False,
                        )
                    nc.tensor.matmul(
                        ps[:], vmean_row[0:1, dtl * 128:(dtl + 1) * 128], fill16[:],
                        start=False, stop=True,
                    )
                    nc.vector.scalar_tensor_tensor(
                        ctxh[:, dtl * 512:(dtl + 1) * 512], ps[:], 1.0,
                        resid16[:, dtl * 512:(dtl + 1) * 512], ALU.mult, ALU.add,
                    )
                if stage == 5:
                    nc.sync.dma_start(out_d[:, :], ctxh[0:1, 0:16].bitcast(BF16))
                    return
                for cls in range(NCLS):
                    scr = (spoolA if cls % 2 else spoolB).tile([128, N], BF16, tag="scrD")
                    nc.vector.scalar_tensor_tensor(
                        scr[:], ctxh[:], 1.0, wf[cls][:], ALU.mult, ALU.mult,
                        accum_out=facc[:, cls:cls + 1],
                    )
            with tc.tile_pool(name="psO", bufs=1, space="PSUM") as psO:
                o = psO.tile([1, 16], F32, tag="o")
                nc.tensor.matmul(o[:], onesr32[:], facc[:], start=True, stop=True)
                osb = smpool.tile([1, 16], F32, tag="osb")
                nc.scalar.copy(osb[:], o[:])
                nc.sync.dma_start(out_d[:, :], osb[:])

    with tile.TileContext(nc) as tc:
        emit(tc)
    nc.compile()
    return nc


_NC_CACHE = {}


def get_nc(stage=9):
    if stage not in _NC_CACHE:
        _NC_CACHE[stage] = build_nc(stage)
    return _NC_CACHE[stage]


def host_prep(inputs):
    """Build per-core input maps from the full problem inputs (layout only)."""
    x = np.asarray(inputs["input_embedding"], np.float32)        # [B, N, D]
    wq = np.asarray(inputs["Wq"], np.float32)
    wk = np.asarray(inputs["Wk"], np.float32)
    wv = np.asarray(inputs["Wv"], np.float32)
    wa = np.asarray(inputs["Wadd"], np.float32)
    badd = np.asarray(inputs["badd"], np.float32)
    wfin = np.asarray(inputs["Wfin"], np.float32)                # [10, N*D]
    idx = np.asarray(inputs["index_sample"]).astype(np.int64)    # [N, U]
    bf = ml_dtypes.bfloat16

    cnt = np.zeros((N, N), np.float32)
    np.add.at(cnt, (np.arange(N)[:, None], idx), 1.0)

    # Core half h=1 gets the n-axis halves swapped on every n-indexed input
    # (the pipeline is equivariant under a joint permutation of X rows,
    # mask rows+cols, and Wfin columns), so "columns 0:512" is its half.
    perms = [np.arange(N), np.concatenate([np.arange(512, N), np.arange(512)])]
    assert cnt.max() <= 16  # fp8e4m3-exact
    am_h, cnt_h = [], []
    for p in perms:
        cp = cnt[p][:, p]
        am_h.append(np.where(cp > 0, 0.0, NEG).astype(ml_dtypes.float8_e5m2))
        cnt_h.append(cp.astype(ml_dtypes.float8_e4m3))

    # Wfin[c, n*256+d] -> [10, d, n_local] -> [10, 128, 2*512] bf16
    wr = wfin.reshape(NCLS, N, D).transpose(0, 2, 1)             # [10, 256, 1024]
    wr_h = [
        np.ascontiguousarray(
            wr[:, :, perms[h][:512]].reshape(NCLS, 2, 128, 512)
            .transpose(0, 2, 1, 3)
        ).reshape(NCLS, 128, N).astype(bf)
        for h in range(2)
    ]

    w_all = np.stack([w.T.reshape(2, 128, D) for w in (wq, wk, wv, wa)])
    misc = np.zeros((128, 518), np.float32)
    misc[:, 0:512] = np.arange(512, dtype=np.float32)[None, :]
    misc[:, 512:516] = (np.arange(128, dtype=np.float32)[:, None]
                        + 128.0 * np.arange(4, dtype=np.float32)[None, :])
    misc[:, 516] = badd[0:128]
    misc[:, 517] = badd[128:256]
    sel8 = np.zeros((8, 1024), np.float32)
    for r in range(8):
        sel8[r, r * 128:(r + 1) * 128] = 1.0
    consts = {
        "w_all_h": np.ascontiguousarray(w_all.reshape(8, 128, D)).astype(bf),
        "misc": misc,
        "ident32": np.eye(128, dtype=np.float32),
        "identb": np.eye(128, dtype=np.float32).astype(ml_dtypes.float8_e5m2),
        "sel8": sel8,
    }

    in_maps = []
    xt_cache = {}
    for c in range(8):
        b, h = c // 2, c % 2
        m = dict(consts)
        if (b, h) not in xt_cache:
            xp = np.ascontiguousarray(x[b][perms[h]])
            xt_cache[(b, h)] = np.ascontiguousarray(xp.T).astype(bf)
        m["xt_h"] = xt_cache[(b, h)]
        m["am_b"] = am_h[h]
        m["cnt_b"] = cnt_h[h]
        m["wfin_h"] = wr_h[h]
        in_maps.append(m)
    return in_maps


def host_combine(results, inputs):
    bfin = np.asarray(inputs["bfin"], np.float32)
    out = np.zeros((B, NCLS), np.float32)
    for c in range(8):
        b = c // 2
        out[b] += results[c]["out10"].reshape(-1)[0:NCLS]
    return out + bfin[None, :]


def kernel(**inputs):
    nc = get_nc()
    in_maps = host_prep(inputs)
    res = run_bass_kernel_spmd(nc, in_maps, core_ids=list(range(8)))
    return host_combine(res.results, inputs)
